# revision 14
# baseline (speedup 1.0000x reference)
"""Trainium2 Bass kernel for nn_CrossAttentionFusion.

Reference network (per row, B=65536):
    a = audio @ Wa.T + ba                       (256)
    t = text @ Wt.T + bt                        (256)
    a_ctx = (t @ Wv_a.T + bv_a) @ Ow_a.T + ob_a   [seq-1 MHA == value+out proj]
    t_ctx = (a @ Wv_t.T + bv_t) @ Ow_t.T + ob_t
    a_out = LN(a + a_ctx); t_out = LN(t + t_ctx)
    z1 = [a_out, t_out] @ W1.T + b1 ; h1 = gelu(LN1(z1))
    h2 = gelu(h1 @ W2.T + b2)
    out = h2 @ W3.T + b3                        (7)

Strategy: pure data parallel over 8 cores (8192 rows each). Activations are
kept feature-major on chip ([feature -> partition, row -> free]) so every
matmul contracts over the partition dim; only the input tiles are transposed
(PE transpose via bf16 identity). The whole datapath runs in bf16 with f32
PSUM accumulation: inputs are pre-cast and concatenated host-side into one
[B_CORE, 1024] tensor.

Host-side algebra folds the whole pre-LN block into ONE fused matmul:
    [a_pre | t_pre] = [audio | text] @ G.T + C,
    G = [[Wa, Fa@Wt], [Ft@Wa, Wt]]  (512 x 1024),  Fa = Ow_a @ Wv_a etc.
LN gamma/beta fold into W1/b1.  The LN mean subtraction for the first two
LNs is POSTPONED through W1: with y' = z * (1/sigma) broadcast,
    z1 = W1f @ y' - s1a (x) (mu_a/sigma_a) - s1t (x) (mu_t/sigma_t) + b1',
where s1a/s1t are per-output-row sums of W1f columns (host-precomputed) and
the (x) terms are rank-1 PE matmuls accumulated inside the z1 chain.  Only
LN1 (followed by the nonlinear gelu) subtracts its mean explicitly via a
partition-broadcast.  1/sigma rows are broadcast to SBUF with the Pool
engine's partition_broadcast, so PSUM stays within 8 banks and every
accumulation group is closed before it is read.  The output is written
feature-major [7, B_CORE] (single contiguous DMA) and transposed on host.
"""
import json

import numpy as np

B, AD, TD, D, NC_OUT = 65536, 256, 768, 256, 7
XD = AD + TD                   # 1024 packed input features
EPS = 1e-5
N_CORES = 8
B_CORE = B // N_CORES          # 8192 rows per core
R = 512                        # rows per tile (moving free dim)
NT = B_CORE // R               # 16 tiles per core
RC = R // 128                  # 4 row chunks of 128


def _split_waits(nc, limit_default=1, limit_matmul=1, nop_limit=1):
    """Walrus in this container allows very few sync waits per instruction.

    Engines issue in order, so excess on_wait entries can be hoisted onto
    NoOps inserted immediately before the overloaded instruction.
    """
    orig = nc.to_json_bytes

    def patched():
        m = json.loads(orig())
        counter = [0]
        for fn in m.get("functions", []):
            for blk in fn.get("blocks", []):
                insts = blk.get("instructions")
                if not insts:
                    continue
                out = []
                for inst in insts:
                    si = inst.get("sync_info")
                    waits = (si or {}).get("on_wait") or []
                    opc = inst.get("opcode", "")
                    limit = (
                        limit_matmul
                        if opc in ("Matmult", "Ldweights")
                        else limit_default
                    )
                    if len(waits) > limit:
                        keep = waits[:limit] if limit > 0 else []
                        hoist = waits[limit:] if limit > 0 else waits
                        for i in range(0, len(hoist), nop_limit):
                            counter[0] += 1
                            out.append({
                                "debug": inst.get("debug", 0),
                                "engine": inst["engine"],
                                "ins": [],
                                "name": f"waitsplit-{counter[0]}",
                                "opcode": "NoOp",
                                "outs": [],
                                "sync_info": {
                                    "on_update": [],
                                    "on_wait": hoist[i:i + nop_limit],
                                },
                            })
                        si["on_wait"] = keep
                    out.append(inst)
                blk["instructions"] = out
        return json.dumps(m).encode()

    nc.to_json_bytes = patched


def _build_program(has_c=False, has_b1=False):
    import concourse.bass as bass
    import concourse.mybir as mybir
    import concourse.tile as tile

    F32 = mybir.dt.float32
    BF16 = mybir.dt.bfloat16
    AF = mybir.ActivationFunctionType

    nc = bass.Bass()

    x = nc.dram_tensor("x", [B_CORE, XD], BF16, kind="ExternalInput")
    # fused first-stage weights, lhsT layout [K=1024, M=512]
    g = nc.dram_tensor("g", [XD, 2 * D], BF16, kind="ExternalInput")
    w1 = nc.dram_tensor("w1", [2 * D, D], BF16, kind="ExternalInput")
    w2 = nc.dram_tensor("w2", [D, D // 2], BF16, kind="ExternalInput")
    w3 = nc.dram_tensor("w3", [D // 2, NC_OUT], BF16, kind="ExternalInput")
    ident = nc.dram_tensor("ident", [128, 128], BF16, kind="ExternalInput")
    onescol = nc.dram_tensor("onescol", [128, 1], BF16, kind="ExternalInput")
    onesrow = nc.dram_tensor("onesrow", [1, R], BF16, kind="ExternalInput")
    # bf16 rank-1 stationaries, one per row:
    # 0,1: -s1a chunks   2,3: -s1t chunks  (column sums of W1f halves)
    # 4,5: C_A chunks    6,7: C_T chunks   8,9: b1' chunks  (constant accum)
    srow = nc.dram_tensor("srow", [1, 10 * 128], BF16, kind="ExternalInput")
    # f32 per-feature constant columns:
    # 0,1: gamma1   2,3: beta1   4: b2   5: b3 (7 partitions)   6: eps
    NV = 7
    vecs = nc.dram_tensor("vecs", [128, NV], F32, kind="ExternalInput")
    out = nc.dram_tensor("out", [NC_OUT, B_CORE], F32, kind="ExternalOutput")

    with tile.TileContext(nc) as tc:
        with (
            tc.tile_pool(name="wsb", bufs=1) as wsb,
            tc.tile_pool(name="io", bufs=1) as io,
            tc.tile_pool(name="act", bufs=1) as act,
            tc.tile_pool(name="ps", bufs=1, space="PSUM") as ps,
        ):
            # ---- persistent weights / constants ----
            g_sb = wsb.tile([128, XD // 128, 2 * D], BF16)
            nc.sync.dma_start(g_sb[:], g.rearrange("(k p) m -> p k m", p=128))
            w1_sb = wsb.tile([128, 2 * D // 128, D], BF16)
            nc.sync.dma_start(w1_sb[:], w1.rearrange("(k p) m -> p k m", p=128))
            w2_sb = wsb.tile([128, D // 128, D // 2], BF16)
            nc.sync.dma_start(w2_sb[:], w2.rearrange("(k p) m -> p k m", p=128))
            w3_sb = wsb.tile([128, NC_OUT], BF16)
            nc.sync.dma_start(w3_sb[:], w3[:])
            id_sb = wsb.tile([128, 128], BF16)
            nc.sync.dma_start(id_sb[:], ident[:])
            oc_sb = wsb.tile([128, 1], BF16)
            nc.sync.dma_start(oc_sb[:], onescol[:])
            or_sb = wsb.tile([1, R], BF16)
            nc.sync.dma_start(or_sb[:], onesrow[:])
            s_sb = wsb.tile([1, 10 * 128], BF16)
            nc.sync.dma_start(s_sb[:], srow[:])

            def sr(i):
                return s_sb[0:1, 128 * i:128 * (i + 1)]
            v_sb = wsb.tile([128, NV], F32)
            nc.sync.dma_start(v_sb[:], vecs[:])
            # persistent output accumulator [7, B_CORE] f32
            o_all = wsb.tile([NC_OUT, B_CORE], F32)

            def vcol(i, np_=128):
                return v_sb[0:np_, i:i + 1]

            _ln_counter = [0]

            def ln_stats(z_ps, tag, want_mu):
                """Stats over partitions of the closed psum chunks z_ps.

                Returns (inv_b, aux): inv_b = [128,R] bf16 SBUF broadcast of
                1/sqrt(var+eps).  aux is w = mu/sd as a [1,R] bf16 row when
                want_mu is False (for postponed subtraction), else a [128,R]
                bf16 SBUF broadcast of mu (for explicit subtraction).
                """
                _ln_counter[0] += 1
                uid = f"{tag}_{_ln_counter[0]}"
                nchunk = len(z_ps)
                xs = []
                for m in range(nchunk):
                    t = act.tile([128, R], BF16, tag="xs", bufs=6,
                                 name=f"xs_{uid}_{m}")
                    if m % 2 == 0:
                        nc.vector.tensor_copy(t[:], z_ps[m][:])
                    else:
                        nc.scalar.copy(t[:], z_ps[m][:])
                    xs.append(t)
                sq = []
                for m in range(nchunk):
                    s = act.tile([128, R], BF16, tag="sq", bufs=6,
                                 name=f"sq_{uid}_{m}")
                    nc.gpsimd.tensor_mul(s[:], xs[m][:], xs[m][:])
                    sq.append(s)
                # partition-reduced stats (oc = 1/256 -> E[x], E[x^2]);
                # both rows live in ONE psum bank (base partitions 0 / 32)
                st = ps.tile([33, R], F32, tag="st", bufs=2,
                             name=f"st_{uid}")
                s_sum = st[0:1, :]
                s_sq = st[32:33, :]
                for m in range(nchunk):
                    nc.tensor.matmul(s_sum, oc_sb[:], xs[m][:],
                                     start=(m == 0), stop=(m == nchunk - 1))
                for m in range(nchunk):
                    nc.tensor.matmul(s_sq, oc_sb[:], sq[m][:],
                                     start=(m == 0), stop=(m == nchunk - 1))
                mu_s = act.tile([1, R], F32, tag="mus", bufs=2,
                                name=f"mus_{uid}")
                nc.vector.tensor_copy(mu_s[:], s_sum)
                musq = act.tile([1, R], F32, tag="musq", bufs=2,
                                name=f"musq_{uid}")
                nc.gpsimd.tensor_mul(musq[:], mu_s[:], mu_s[:])
                var = act.tile([1, R], F32, tag="var", bufs=2, name=f"var_{uid}")
                nc.vector.tensor_sub(var[:], s_sq, musq[:])
                sd = act.tile([1, R], F32, tag="sd", bufs=2, name=f"sd_{uid}")
                nc.scalar.activation(sd[:], var[:], AF.Sqrt, bias=vcol(6, 1))
                inv = act.tile([1, R], BF16, tag="inv", bufs=2, name=f"inv_{uid}")
                with nc.allow_low_precision(reason="bf16 rounding of 1/sd"):
                    nc.vector.reciprocal(inv[:], sd[:])
                # PE rank-1 broadcast into the shared psum slot, then copy
                # to SBUF bf16 so the bank frees immediately
                bc = ps.tile([128, R], F32, tag="bc", bufs=1,
                             name=f"bc_{uid}")
                nc.tensor.matmul(bc[:], or_sb[:, 0:128], inv[:],
                                 start=True, stop=True)
                inv_b = act.tile([128, R], BF16, tag="invb", bufs=3,
                                 name=f"invb_{uid}")
                nc.vector.tensor_copy(inv_b[:], bc[:])
                if want_mu:
                    mu_r = act.tile([1, R], BF16, tag="mur", bufs=2,
                                    name=f"mur_{uid}")
                    with nc.allow_low_precision(reason="bf16 mu row"):
                        nc.gpsimd.tensor_copy(mu_r[:], mu_s[:])
                    bc2 = ps.tile([128, R], F32, tag="bc", bufs=1,
                                  name=f"bc2_{uid}")
                    nc.tensor.matmul(bc2[:], or_sb[:, 0:128], mu_r[:],
                                     start=True, stop=True)
                    mu_b = act.tile([128, R], BF16, tag="mub", bufs=2,
                                    name=f"mub_{uid}")
                    nc.vector.tensor_copy(mu_b[:], bc2[:])
                    return inv_b, mu_b
                else:
                    w = act.tile([1, R], BF16, tag="w", bufs=4,
                                 name=f"w_{uid}")
                    with nc.allow_low_precision(reason="bf16 mu/sd row"):
                        nc.gpsimd.tensor_mul(w[:], mu_s[:], inv[:])
                    return inv_b, w

            # ---------------- main loop over row tiles ----------------
            for it in range(NT):
                r0 = it * R
                # natural load [128, RC, XD] bf16 (rows 4x2KB contiguous)
                x_nat = io.tile([128, RC, XD], BF16, tag="x_nat", bufs=2,
                                name=f"x_nat_{it}")
                nc.sync.dma_start(
                    x_nat[:], x[r0:r0 + R, :].rearrange("(c p) f -> p c f", p=128))

                # PE transpose -> feature-major bf16 tiles (8 chunks of 128)
                xT = []
                for fc in range(XD // 128):
                    pt = ps.tile([128, R], BF16, tag="tr", bufs=1,
                                 name=f"pt_{it}_{fc}")
                    for c in range(RC):
                        nc.tensor.transpose(
                            pt[:, 128 * c:128 * (c + 1)],
                            x_nat[:, c, 128 * fc:128 * (fc + 1)],
                            id_sb[:])
                    tr = act.tile([128, R], BF16, tag="xT", bufs=XD // 128 + 2,
                                  name=f"xT_{it}_{fc}")
                    if fc % 8 in (2, 5, 7):
                        nc.scalar.copy(tr[:], pt[:])
                    else:
                        nc.vector.tensor_copy(tr[:], pt[:])
                    xT.append(tr)

                # [a_pre | t_pre] = x @ G.T (+ C rank-1), single fused chain
                zp = [ps.tile([128, R], F32, tag="acc", bufs=4,
                              name=f"zp_{it}_{mc}") for mc in range(4)]
                for mc in range(4):
                    last = 7 if not has_c else -1
                    for k in range(XD // 128):
                        nc.tensor.matmul(zp[mc][:],
                                         g_sb[:, k, 128 * mc:128 * (mc + 1)],
                                         xT[k][:], start=(k == 0),
                                         stop=(k == last))
                    if has_c:
                        ci = 4 + mc if mc < 2 else 6 + (mc - 2)
                        nc.tensor.matmul(zp[mc][:], sr(ci),
                                         or_sb[:], start=False, stop=True)
                pa_ps, pt_ps = zp[:2], zp[2:]

                inv_a, w_a = ln_stats(pa_ps, "lna", want_mu=False)
                inv_t, w_t = ln_stats(pt_ps, "lnt", want_mu=False)

                # y' = z * inv  (mean handled via rank-1s in the z1 chain)
                y = []
                for src, invb, tg in ((pa_ps, inv_a, "ya"), (pt_ps, inv_t, "yt")):
                    for m in range(2):
                        o = act.tile([128, R], BF16, tag="y", bufs=8,
                                     name=f"y_{tg}_{it}_{m}")
                        nc.vector.tensor_mul(o[:], src[m][:], invb[:])
                        y.append(o)

                # z1 = y' @ W1f.T - s1a (x) w_a - s1t (x) w_t (+ b1' rank-1)
                z1_ps = [ps.tile([128, R], F32, tag="acc", bufs=4,
                                 name=f"z1ps_{it}_{m}") for m in range(2)]
                for m in range(2):
                    for k in range(4):
                        nc.tensor.matmul(z1_ps[m][:],
                                         w1_sb[:, k, 128 * m:128 * (m + 1)],
                                         y[k][:], start=(k == 0), stop=False)
                    nc.tensor.matmul(z1_ps[m][:], sr(0 + m),
                                     w_a[:], start=False, stop=False)
                    nc.tensor.matmul(z1_ps[m][:], sr(2 + m),
                                     w_t[:], start=False,
                                     stop=not has_b1)
                    if has_b1:
                        nc.tensor.matmul(z1_ps[m][:], sr(8 + m),
                                         or_sb[:], start=False, stop=True)

                inv_1, mu_1 = ln_stats(z1_ps, "ln1", want_mu=True)
                h1 = []
                for m in range(2):
                    d = act.tile([128, R], BF16, tag="d1", bufs=4,
                                 name=f"d1_{it}_{m}")
                    nc.vector.tensor_sub(d[:], z1_ps[m][:], mu_1[:])
                    e = act.tile([128, R], BF16, tag="e1", bufs=4,
                                 name=f"e1_{it}_{m}")
                    nc.gpsimd.tensor_mul(e[:], d[:], inv_1[:])
                    h = act.tile([128, R], BF16, tag="h1", bufs=4,
                                 name=f"h1_{it}_{m}")
                    nc.scalar.activation(h[:], e[:], AF.Gelu,
                                         bias=vcol(2 + m), scale=vcol(m))
                    h1.append(h)

                # h2 = gelu(h1 @ W2.T + b2)   (128 features -> 1 chunk)
                z2_ps = ps.tile([128, R], F32, tag="acc", bufs=4,
                                name=f"z2ps_{it}")
                for k in range(2):
                    nc.tensor.matmul(z2_ps[:], w2_sb[:, k, :], h1[k][:],
                                     start=(k == 0), stop=(k == 1))
                h2 = act.tile([128, R], BF16, tag="h2", bufs=2,
                              name=f"h2_{it}")
                nc.scalar.activation(h2[:], z2_ps[:], AF.Gelu, bias=vcol(4))

                # out chunk = h2 @ W3.T + b3 -> [7, R] into o_all columns
                z3_ps = ps.tile([NC_OUT, R], F32, tag="st", bufs=2,
                                name=f"z3ps_{it}")
                nc.tensor.matmul(z3_ps[:], w3_sb[:], h2[:], start=True,
                                 stop=True)
                nc.vector.tensor_scalar_add(o_all[:, r0:r0 + R], z3_ps[:],
                                            v_sb[0:NC_OUT, 5:6])

            nc.sync.dma_start(out[:], o_all[:])

    _split_waits(nc)
    return nc


def _host_weights(Wa, ba, Wt, bt, a2t_in_w, a2t_in_b, a2t_out_w, a2t_out_b,
                  t2a_in_w, t2a_in_b, t2a_out_w, t2a_out_b,
                  ln_a_g, ln_a_b, ln_t_g, ln_t_b, W1, b1, ln1_g, ln1_b,
                  W2, b2, W3, b3):
    import ml_dtypes
    f8 = np.float64
    bf = ml_dtypes.bfloat16
    Wv_a = a2t_in_w[2 * D:].astype(f8)
    bv_a = a2t_in_b[2 * D:].astype(f8)
    Wv_t = t2a_in_w[2 * D:].astype(f8)
    bv_t = t2a_in_b[2 * D:].astype(f8)
    # a_ctx = t_full @ Fa.T + c_ma with Fa = Ow_a @ Wv_a
    Fa = a2t_out_w.astype(f8) @ Wv_a
    c_ma = bv_a @ a2t_out_w.astype(f8).T + a2t_out_b.astype(f8)
    Ft = t2a_out_w.astype(f8) @ Wv_t
    c_mt = bv_t @ t2a_out_w.astype(f8).T + t2a_out_b.astype(f8)
    # a_pre = audio@Wa.T + text@(Fa@Wt).T + C_A
    # t_pre = audio@(Ft@Wa).T + text@Wt.T + C_T
    C_A = ba.astype(f8) + bt.astype(f8) @ Fa.T + c_ma
    C_T = bt.astype(f8) + ba.astype(f8) @ Ft.T + c_mt

    # fused first stage G.T as lhsT [1024, 512]
    G = np.zeros((XD, 2 * D), f8)
    G[:AD, :D] = Wa.astype(f8).T
    G[AD:, :D] = (Fa @ Wt.astype(f8)).T
    G[:AD, D:] = (Ft @ Wa.astype(f8)).T
    G[AD:, D:] = Wt.astype(f8).T

    # fold LN_a/LN_t gamma+beta into W1/b1:
    # z1 = (y*Gam + Bet) @ W1.T + b1 = y @ (W1*Gam).T + (Bet @ W1.T + b1)
    Gam = np.concatenate([np.asarray(ln_a_g, f8), np.asarray(ln_t_g, f8)])
    Bet = np.concatenate([np.asarray(ln_a_b, f8), np.asarray(ln_t_b, f8)])
    W1f = W1.astype(f8) * Gam[None, :]
    b1f = b1.astype(f8) + Bet @ W1.astype(f8).T
    # postponed-mean rank-1 stationaries: -sum over each input half of W1f
    s1a = -W1f[:, :D].sum(axis=1)       # (256,)
    s1t = -W1f[:, D:].sum(axis=1)

    has_c = bool(np.any(C_A != 0) or np.any(C_T != 0))
    has_b1 = bool(np.any(b1f != 0))

    def col(v, chunk):
        return np.asarray(v, np.float32)[128 * chunk:128 * (chunk + 1)].reshape(128, 1)

    NV = 7
    vecs = np.zeros((128, NV), np.float32)
    for c in range(2):
        vecs[:, 0 + c:1 + c] = col(ln1_g, c)
        vecs[:, 2 + c:3 + c] = col(ln1_b, c)
    vecs[:, 4:5] = np.asarray(b2, np.float32).reshape(128, 1)
    vecs[0:NC_OUT, 5] = np.asarray(b3, np.float32)
    vecs[:, 6] = EPS

    srow = np.zeros((10, 128), np.float32)
    for c in range(2):
        srow[0 + c] = col(s1a, c).ravel()
        srow[2 + c] = col(s1t, c).ravel()
        srow[4 + c] = col(C_A, c).ravel()
        srow[6 + c] = col(C_T, c).ravel()
        srow[8 + c] = col(b1f, c).ravel()
    srow = srow.reshape(1, 10 * 128)

    wmap = {
        "g": np.ascontiguousarray(G).astype(bf),
        "w1": np.ascontiguousarray(W1f.T).astype(bf),
        "w2": np.ascontiguousarray(W2.T).astype(bf),
        "w3": np.ascontiguousarray(W3.T).astype(bf),
        "ident": np.eye(128, dtype=np.float32).astype(bf),
        "onescol": np.full((128, 1), 1.0 / 256, np.float32).astype(bf),
        "onesrow": np.ones((1, R), np.float32).astype(bf),
        "srow": srow.astype(bf),
        "vecs": vecs,
    }
    return wmap, has_c, has_b1


def _pack_x(audio, text):
    """Concatenate audio|text along features and cast to bf16."""
    import ml_dtypes
    bf = ml_dtypes.bfloat16
    x = np.empty((B, XD), dtype=bf)
    x[:, :AD] = audio.astype(bf)
    x[:, AD:] = text.astype(bf)
    return x


_PROGRAM_CACHE = {}


def _get_program(has_c, has_b1):
    key = (has_c, has_b1)
    if key not in _PROGRAM_CACHE:
        _PROGRAM_CACHE[key] = _build_program(has_c, has_b1)
    return _PROGRAM_CACHE[key]


def kernel(**inputs):
    inputs = {k: np.asarray(v) for k, v in inputs.items()}
    audio = np.ascontiguousarray(inputs["audio_vec"], np.float32)
    text = np.ascontiguousarray(inputs["text_vec"], np.float32)
    wmap, has_c, has_b1 = _host_weights(
        **{k: np.asarray(v) for k, v in inputs.items()
           if k not in ("audio_vec", "text_vec")})
    xall = _pack_x(audio, text)

    nc = _get_program(has_c, has_b1)

    from concourse.bass_utils import run_bass_kernel_spmd

    in_maps = []
    for c in range(N_CORES):
        m = dict(wmap)
        m["x"] = xall[c * B_CORE:(c + 1) * B_CORE]
        in_maps.append(m)

    res = run_bass_kernel_spmd(nc, in_maps, core_ids=list(range(N_CORES)))
    out = np.concatenate(
        [np.asarray(res.results[c]["out"]).T for c in range(N_CORES)], axis=0)
    return np.ascontiguousarray(out, np.float32)


if __name__ == "__main__":
    rng = np.random.default_rng(0)
    ins = {
        "audio_vec": rng.standard_normal((B, AD), dtype=np.float32),
        "text_vec": rng.standard_normal((B, TD), dtype=np.float32),
    }
    print(kernel(**ins).shape)
